# revision 22
# baseline (speedup 1.0000x reference)
"""Trainium2 Bass kernel for nn_AttentionLayer (sparse_attention).

Reference computation (B=4, N=2048, C=256, H=8, HD=32):
    qkv = x @ qkv_w.T; q,k,v = split(qkv); heads
    scores = q k^T / sqrt(HD) + adj          [B,H,N,N]
    out    = softmax(scores) @ v             -> merge heads [B,N,C]
    result = (out*0.1 + x) @ out_w.T + out_b
(The pos_proj(adj) value in the reference is dead code; x0 is unused.)

Sharding: 8 cores = (batch b, query-half).  Core c handles batch c//2 and
query rows [(c%2)*1024, (c%2+1)*1024).  Each core computes K/V for its
whole batch locally (no collectives); the host rolls the key axis so the
core's own query rows are rows 0..1023 of its x input, and rolls adj
columns the same way (softmax is key-permutation invariant).

Per-core pipeline (engine assignment tuned against NTFF traces):
  scores (bf16 PE matmuls, SCALE folded into host-side q weights) land
  in PSUM [key, query]; per (head-pair, key-tile) either
    add path:  DVE adds adj (fp32) for both heads into a shared
               [128,2,1024] tile, one ACT exp covers the pair (bf16 out)
    pool path (POOL_KTS): ACT exps raw scores straight from PSUM and
               GPSIMD multiplies by precomputed exp(adj)
               (exp(a+b) = exp(a)exp(b)) - offloads DVE/ACT onto the
               otherwise idle Pool engine.
  attention out = E @ [v | 10]: row base+32 of po is 10*sum(E), giving
  the softmax denominator with the 0.1 output scale folded in.  The
  denominator reciprocal is computed on a DMA-spread [64,32] layout
  (DVE RECIPROCAL cost scales with free size: [1,1024] is ~7.9us, the
  spread form ~0.5us), broadcast back, and applied on the head's rows
  of attT.  q/k/v live in 3-head stacks (base partitions 0/32/64) so
  consecutive score matmuls of a head pair sit on different PE row
  quads; attnv writes po halves at PSUM partitions 0/64.
"""

import sys

for _p in ("/opt/trn_rl_repo", "/root/.axon_site/_ro/trn_rl_repo"):
    if _p not in sys.path:
        sys.path.insert(0, _p)

import ml_dtypes
import numpy as np

import concourse.mybir as mybir
from concourse import bacc
from concourse.bass import ds, ts
from concourse.tile import TileContext

B, N, C, H = 4, 2048, 256, 8
HD = C // H          # 32
NQ = N // 2          # 1024 query rows per core
SCALE = 1.0 / np.sqrt(HD)
FP32 = mybir.dt.float32
BF16 = mybir.dt.bfloat16
P = 128
BF16NP = ml_dtypes.bfloat16

_CACHED = {}

POOL_KTS = (2, 5, 8, 10, 15)   # key tiles on the ACT-exp + GPSIMD-mult path
PAIRS = ((0, 2), (1, 3), (4, 6), (5, 7))


def build_kernel(repeat=1):
    nc = bacc.Bacc("TRN2", target_bir_lowering=False)
    xt_ext = nc.declare_dram_parameter("xt", [C, N], BF16, isOutput=False)
    adjf_ext = nc.declare_dram_parameter("adjf", [P, 16, NQ], FP32, isOutput=False)
    wt_ext = nc.declare_dram_parameter("qkv_wt", [C, 3 * C], BF16, isOutput=False)
    owt_ext = nc.declare_dram_parameter("out_wt", [C, C], BF16, isOutput=False)
    outb_ext = nc.declare_dram_parameter("out_b", [P, C], FP32, isOutput=False)
    out_ext = nc.declare_dram_parameter("out", [NQ, C], FP32, isOutput=True)

    with TileContext(nc) as tc:
        with (
            tc.tile_pool(name="const", bufs=1) as constp,
            tc.tile_pool(name="persist", bufs=1) as persist,
            tc.tile_pool(name="work", bufs=2) as work,
            tc.tile_pool(name="sp_pool", bufs=3, space="PSUM") as spp,
            tc.tile_pool(name="po_pool", bufs=1, space="PSUM") as pop,
        ):
            outb_bc = constp.tile([P, C], FP32)
            nc.sync.dma_start(outb_bc[:], outb_ext[:, :])
            for _ in range(repeat):
                _body(nc, tc, persist, work, spp, pop, outb_bc,
                      xt_ext, adjf_ext, wt_ext, owt_ext, out_ext)

    nc.compile()
    return nc


def _body(nc, tc, persist, work, spp, pop, outb_bc,
          xt_ext, adjf_ext, wt_ext, owt_ext, out_ext):
    AF = mybir.ActivationFunctionType
    ALU = mybir.AluOpType

    # ---------------- persistent SBUF tensors ----------------
    xT = [persist.tile([P, N], BF16, tag=f"xT{i}", name=f"xT{i}") for i in range(2)]
    wT = [persist.tile([P, 3 * C], BF16, tag=f"wT{i}", name=f"wT{i}") for i in range(2)]
    owT = [persist.tile([P, C], BF16, tag=f"owT{i}", name=f"owT{i}") for i in range(2)]
    # head stacks: head h lives in stack h//3 at rows 32*(h%3)
    kS = [persist.tile([P, N], BF16, tag=f"kS{i}", name=f"kS{i}") for i in range(3)]
    qS = [persist.tile([P, NQ], BF16, tag=f"qS{i}", name=f"qS{i}") for i in range(3)]
    vv = persist.tile([P, 16, H, HD + 1], BF16, tag="vv")
    adjF = persist.tile([P, 16, NQ], FP32, tag="adjF")
    eadjT = persist.tile([P, len(POOL_KTS), NQ], BF16, tag="eadjT")
    attT = [persist.tile([P, NQ], BF16, tag=f"attT{i}", name=f"attT{i}")
            for i in range(2)]

    # ---------------- loads (already transposed/bf16 on host) -------------
    for j in range(2):
        for hseg in range(2):
            nc.sync.dma_start(xT[j][:, ds(hseg * NQ, NQ)],
                              xt_ext[ds(j * P, P), ds(hseg * NQ, NQ)])
        nc.sync.dma_start(wT[j][:], wt_ext[ds(j * P, P), :])
        nc.sync.dma_start(owT[j][:], owt_ext[ds(j * P, P), :])
    for i in range(16):
        nc.gpsimd.dma_start(adjF[:, i, :], adjf_ext[:, i, :])

    # eadj = exp(adj) for the pool-path key tiles (ACT, overlaps qkv)
    for i, kt in enumerate(POOL_KTS):
        nc.scalar.activation(eadjT[:, i, :], adjF[:, kt, :], AF.Exp)

    # ---------------- QKV projections (bf16, col-pair tiled) --------------
    def proj_stack(dst, off, nchs):
        for j in range(3):
            w0 = 96 * j
            for nch in range(nchs):
                pp = spp.tile([P, NQ], FP32, tag="sp", name="pp")[:, :512]
                for (cb, cw) in ((0, 64), (64, 32)) if j < 2 else ((0, 64),):
                    for cc in range(2):
                        nc.tensor.matmul(pp[ds(cb, cw), :],
                                         wT[cc][:, ds(off + w0 + cb, cw)],
                                         xT[cc][:, ts(nch, 512)],
                                         start=(cc == 0), stop=(cc == 1))
                rows = 96 if j < 2 else 64
                nc.vector.tensor_copy(dst[j][ds(0, rows), ts(nch, 512)],
                                      pp[ds(0, rows), :])

    proj_stack(qS, 0, 2)
    proj_stack(kS, C, 4)
    # v: [key_tile, head, hd] with ones column scaled by 10 (folds the 0.1)
    nc.vector.memset(vv[:, :, :, HD], 10.0)
    for kt in range(16):
        pv = spp.tile([P, NQ], FP32, tag="sp", name="pv")[:, :512]
        for cc in range(2):
            nc.tensor.matmul(pv[:, :C], xT[cc][:, ts(kt, P)],
                             wT[cc][:, ds(2 * C, C)],
                             start=(cc == 0), stop=(cc == 1))
        nc.vector.tensor_copy(
            vv[:, kt, :, 0:HD],
            pv[:, :C].rearrange("p (h d) -> p h d", h=H))

    # ---------------- attention: head pairs ----------------
    for hA, hB in PAIRS:
        sA, rA = hA // 3, 32 * (hA % 3)
        sB, rB = hB // 3, 32 * (hB % 3)
        po = pop.tile([P, NQ], FP32, tag="po", name="po")
        natt = [0]

        def attnv(kt, E2):
            natt[0] += 1
            for c in range(2):
                nc.tensor.matmul(po[ds(0, HD + 1), ds(512 * c, 512)],
                                 vv[:, kt, hA, :],
                                 E2[:, 0, ds(512 * c, 512)],
                                 start=(natt[0] == 1), stop=(natt[0] == 16),
                                 skip_group_check=True)
                nc.tensor.matmul(po[ds(64, HD + 1), ds(512 * c, 512)],
                                 vv[:, kt, hB, :],
                                 E2[:, 1, ds(512 * c, 512)],
                                 start=(natt[0] == 1), stop=(natt[0] == 16),
                                 skip_group_check=True)

        for kt in range(16):
            spA = spp.tile([P, NQ], FP32, tag="sp", name="spA")
            spB = spp.tile([P, NQ], FP32, tag="sp", name="spB")
            for c in range(2):
                nc.tensor.matmul(spA[:, ds(512 * c, 512)],
                                 kS[sA][ds(rA, HD), ts(kt, P)],
                                 qS[sA][ds(rA, HD), ds(512 * c, 512)],
                                 start=True, stop=True)
                nc.tensor.matmul(spB[:, ds(512 * c, 512)],
                                 kS[sB][ds(rB, HD), ts(kt, P)],
                                 qS[sB][ds(rB, HD), ds(512 * c, 512)],
                                 start=True, stop=True)
            if kt in POOL_KTS:
                ea = eadjT[:, POOL_KTS.index(kt), :]
                E2 = work.tile([P, 2, NQ], BF16, tag="E2", name="E2", bufs=6)
                PtA = work.tile([P, NQ], BF16, tag="Pt", name="PtA", bufs=4)
                PtB = work.tile([P, NQ], BF16, tag="Pt", name="PtB", bufs=4)
                nc.scalar.activation(PtA[:], spA[:], AF.Exp)
                nc.gpsimd.tensor_tensor(E2[:, 0, :], PtA[:], ea, ALU.mult)
                nc.scalar.activation(PtB[:], spB[:], AF.Exp)
                nc.gpsimd.tensor_tensor(E2[:, 1, :], PtB[:], ea, ALU.mult)
            else:
                aj = adjF[:, kt, :]
                sm2 = work.tile([P, 2, NQ], FP32, tag="sm2", name="sm2", bufs=3)
                nc.vector.tensor_tensor(sm2[:, 0, :], spA[:], aj, ALU.add)
                nc.vector.tensor_tensor(sm2[:, 1, :], spB[:], aj, ALU.add)
                E2 = work.tile([P, 2, NQ], BF16, tag="E2", name="E2", bufs=6)
                nc.scalar.activation(E2[:], sm2[:], AF.Exp)
            attnv(kt, E2)
        # ---- normalize: rows base..base+31 by 1/(10*sum) ----
        # reciprocal on a DMA-spread [64,32] layout (DVE recip cost scales
        # with free size; [1,1024] is ~7.9us, this form ~0.5us total)
        cpA = work.tile([1, NQ], FP32, tag="cpA", name="cpA")
        cpB = work.tile([1, NQ], FP32, tag="cpB", name="cpB")
        nc.scalar.copy(cpA[:], po[ds(HD, 1), :])
        nc.vector.tensor_copy(cpB[:], po[ds(64 + HD, 1), :])
        dsp = work.tile([64, 32], FP32, tag="dsp", name="dsp")
        for i, cp in enumerate((cpA, cpB)):
            nc.sync.dma_start(dsp[ds(32 * i, 32), :],
                              cp[:, :].rearrange("o (p j) -> o p j", p=32))
        rc = work.tile([64, 32], FP32, tag="rc", name="rc")
        nc.vector.reciprocal(rc[:], dsp[:])
        rr = work.tile([2, NQ], FP32, tag="rr", name="rr")
        for i in range(2):
            nc.sync.dma_start(rr[ds(i, 1), :].rearrange("o (p j) -> o p j", p=32),
                              rc[ds(32 * i, 32), :])
        for i, (h, base) in enumerate(((hA, 0), (hB, 64))):
            bc = work.tile([HD, NQ], FP32, tag="bc", name="bc")
            nc.sync.dma_start(bc[:], rr[ds(i, 1), None, :].to_broadcast((1, HD, NQ)))
            home, chunk = 32 * (h % 4), h // 4
            nc.vector.tensor_tensor(attT[chunk][ds(home, HD), :],
                                    po[ds(base, HD), :], bc[:], ALU.mult)

    # ---------------- residual + out_proj ----------------
    for cc in range(2):
        nc.vector.tensor_tensor(attT[cc][:], attT[cc][:], xT[cc][:, 0:NQ],
                                ALU.add)
    for rt in range(8):
        pf = spp.tile([P, NQ], FP32, tag="sp", name="pf")[:, :C]
        for cc in range(2):
            nc.tensor.matmul(pf[:, :C], attT[cc][:, ts(rt, P)], owT[cc][:],
                             start=(cc == 0), stop=(cc == 1))
        osb = work.tile([P, C], FP32, tag="osb", name="osb")
        nc.vector.tensor_tensor(osb[:], pf[:, :C], outb_bc[:], ALU.add)
        nc.sync.dma_start(out_ext[ds(rt * P, P), :], osb[:])


def _run(nc, in_maps):
    from concourse.bass_utils import run_bass_kernel_spmd
    res = run_bass_kernel_spmd(nc, in_maps, core_ids=list(range(8)))
    return res.results


def make_in_maps(x, adj, qkv_w, out_w, out_b):
    x = np.asarray(x, np.float32)
    adj = np.asarray(adj, np.float32)
    w = np.asarray(qkv_w, np.float32).copy()
    w[:C] *= SCALE                       # fold 1/sqrt(HD) into q weights
    wt = np.ascontiguousarray(w.T).astype(BF16NP)
    owt = np.ascontiguousarray(np.asarray(out_w, np.float32).T).astype(BF16NP)
    outb = np.ascontiguousarray(
        np.broadcast_to(np.asarray(out_b, np.float32), (P, C)))
    in_maps = []
    for c in range(8):
        b, half = divmod(c, 2)
        xb = np.roll(x[b], -half * NQ, axis=0)
        xt = np.ascontiguousarray(xb.T).astype(BF16NP)          # [C, N]
        aj = np.roll(adj[half * NQ:(half + 1) * NQ, :], -half * NQ, axis=1)
        ajf = np.ascontiguousarray(
            aj.T.reshape(16, P, NQ).transpose(1, 0, 2))          # [P, 16, NQ]
        in_maps.append({
            "xt": xt, "adjf": ajf,
            "qkv_wt": wt, "out_wt": owt, "out_b": outb,
        })
    return in_maps


def kernel(x, x0, adj, qkv_w, out_w, out_b, pos_w, pos_b):
    """Full-input, full-output entry point.  x0/pos_w/pos_b are dead in the
    reference computation and are ignored."""
    if "nc" not in _CACHED:
        _CACHED["nc"] = build_kernel(repeat=1)
    nc = _CACHED["nc"]
    in_maps = make_in_maps(x, adj, qkv_w, out_w, out_b)
    results = _run(nc, in_maps)
    out = np.empty((B, N, C), np.float32)
    for c in range(8):
        b, half = divmod(c, 2)
        out[b, half * NQ:(half + 1) * NQ, :] = results[c]["out"]
    return out


# revision 23
# speedup vs baseline: 1.2046x; 1.2046x over previous
"""Trainium2 Bass kernel for nn_AttentionLayer (sparse_attention).

Reference computation (B=4, N=2048, C=256, H=8, HD=32):
    qkv = x @ qkv_w.T; q,k,v = split(qkv); heads
    scores = q k^T / sqrt(HD) + adj          [B,H,N,N]
    out    = softmax(scores) @ v             -> merge heads [B,N,C]
    result = (out*0.1 + x) @ out_w.T + out_b
(The pos_proj(adj) value in the reference is dead code; x0 is unused.)

Sharding: 8 cores = (batch b, query-half).  Core c handles batch c//2 and
query rows [(c%2)*1024, (c%2+1)*1024).  Each core computes K/V for its
whole batch locally (no collectives); the host rolls the key axis so the
core's own query rows are rows 0..1023 of its x input, and rolls adj
columns the same way (softmax is key-permutation invariant).

Per-core pipeline (engine assignment tuned against NTFF traces):
  scores (bf16 PE matmuls, SCALE folded into host-side q weights) land
  in PSUM [key, query]; per (head-pair, key-tile) either
    add path:  DVE adds adj (fp32) for both heads into a shared
               [128,2,1024] tile, one ACT exp covers the pair (bf16 out)
    pool path (POOL_KTS): ACT exps raw scores straight from PSUM and
               GPSIMD multiplies by precomputed exp(adj)
               (exp(a+b) = exp(a)exp(b)) - offloads DVE/ACT onto the
               otherwise idle Pool engine.
  attention out = E @ [v | 10]: row base+32 of po is 10*sum(E), giving
  the softmax denominator with the 0.1 output scale folded in.  The
  denominator reciprocal is computed on a DMA-spread [64,32] layout
  (DVE RECIPROCAL cost scales with free size: [1,1024] is ~7.9us, the
  spread form ~0.5us), broadcast back, and applied on the head's rows
  of attT.  q/k/v live in 3-head stacks (base partitions 0/32/64) so
  consecutive score matmuls of a head pair sit on different PE row
  quads; attnv writes po halves at PSUM partitions 0/64.
"""

import sys

for _p in ("/opt/trn_rl_repo", "/root/.axon_site/_ro/trn_rl_repo"):
    if _p not in sys.path:
        sys.path.insert(0, _p)

import ml_dtypes
import numpy as np

import concourse.mybir as mybir
from concourse import bacc
from concourse.bass import ds, ts
from concourse.tile import TileContext

B, N, C, H = 4, 2048, 256, 8
HD = C // H          # 32
NQ = N // 2          # 1024 query rows per core
SCALE = 1.0 / np.sqrt(HD)
FP32 = mybir.dt.float32
BF16 = mybir.dt.bfloat16
P = 128
BF16NP = ml_dtypes.bfloat16

_CACHED = {}

POOL_KTS = (5, 10, 15)   # key tiles on the ACT-exp + GPSIMD-mult path
PAIRS = ((0, 2), (1, 3), (4, 6), (5, 7))


def build_kernel(repeat=1):
    nc = bacc.Bacc("TRN2", target_bir_lowering=False)
    xt_ext = nc.declare_dram_parameter("xt", [C, N], BF16, isOutput=False)
    adjf_ext = nc.declare_dram_parameter("adjf", [P, 16, NQ], FP32, isOutput=False)
    wt_ext = nc.declare_dram_parameter("qkv_wt", [C, 3 * C], BF16, isOutput=False)
    owt_ext = nc.declare_dram_parameter("out_wt", [C, C], BF16, isOutput=False)
    outb_ext = nc.declare_dram_parameter("out_b", [P, C], FP32, isOutput=False)
    out_ext = nc.declare_dram_parameter("out", [NQ, C], FP32, isOutput=True)

    with TileContext(nc) as tc:
        with (
            tc.tile_pool(name="const", bufs=1) as constp,
            tc.tile_pool(name="persist", bufs=1) as persist,
            tc.tile_pool(name="work", bufs=2) as work,
            tc.tile_pool(name="sp_pool", bufs=3, space="PSUM") as spp,
            tc.tile_pool(name="po_pool", bufs=1, space="PSUM") as pop,
        ):
            outb_bc = constp.tile([P, C], FP32)
            nc.sync.dma_start(outb_bc[:], outb_ext[:, :])
            for _ in range(repeat):
                _body(nc, tc, persist, work, spp, pop, outb_bc,
                      xt_ext, adjf_ext, wt_ext, owt_ext, out_ext)

    nc.compile()
    return nc


def _body(nc, tc, persist, work, spp, pop, outb_bc,
          xt_ext, adjf_ext, wt_ext, owt_ext, out_ext):
    AF = mybir.ActivationFunctionType
    ALU = mybir.AluOpType

    # ---------------- persistent SBUF tensors ----------------
    xT = [persist.tile([P, N], BF16, tag=f"xT{i}", name=f"xT{i}") for i in range(2)]
    wT = [persist.tile([P, 3 * C], BF16, tag=f"wT{i}", name=f"wT{i}") for i in range(2)]
    owT = [persist.tile([P, C], BF16, tag=f"owT{i}", name=f"owT{i}") for i in range(2)]
    # head stacks: head h lives in stack h//3 at rows 32*(h%3)
    kS = [persist.tile([P, N], BF16, tag=f"kS{i}", name=f"kS{i}") for i in range(3)]
    qS = [persist.tile([P, NQ], BF16, tag=f"qS{i}", name=f"qS{i}") for i in range(3)]
    vv = persist.tile([P, 16, H, HD + 1], BF16, tag="vv")
    adjF = persist.tile([P, 16, NQ], FP32, tag="adjF")
    eadjT = persist.tile([P, len(POOL_KTS), NQ], BF16, tag="eadjT")
    attT = [persist.tile([P, NQ], BF16, tag=f"attT{i}", name=f"attT{i}")
            for i in range(2)]

    # ---------------- loads (already transposed/bf16 on host) -------------
    for j in range(2):
        for hseg in range(2):
            nc.sync.dma_start(xT[j][:, ds(hseg * NQ, NQ)],
                              xt_ext[ds(j * P, P), ds(hseg * NQ, NQ)])
        nc.sync.dma_start(wT[j][:], wt_ext[ds(j * P, P), :])
        nc.sync.dma_start(owT[j][:], owt_ext[ds(j * P, P), :])
    for i in range(16):
        nc.gpsimd.dma_start(adjF[:, i, :], adjf_ext[:, i, :])

    # eadj = exp(adj) for the pool-path key tiles (ACT, overlaps qkv)
    for i, kt in enumerate(POOL_KTS):
        nc.scalar.activation(eadjT[:, i, :], adjF[:, kt, :], AF.Exp)

    # ---------------- QKV projections (bf16, col-pair tiled) --------------
    def proj_stack(dst, off, nchs):
        for j in range(3):
            w0 = 96 * j
            for nch in range(nchs):
                pp = spp.tile([P, NQ], FP32, tag="sp", name="pp")[:, :512]
                for (cb, cw) in ((0, 64), (64, 32)) if j < 2 else ((0, 64),):
                    for cc in range(2):
                        nc.tensor.matmul(pp[ds(cb, cw), :],
                                         wT[cc][:, ds(off + w0 + cb, cw)],
                                         xT[cc][:, ts(nch, 512)],
                                         start=(cc == 0), stop=(cc == 1))
                rows = 96 if j < 2 else 64
                nc.vector.tensor_copy(dst[j][ds(0, rows), ts(nch, 512)],
                                      pp[ds(0, rows), :])

    proj_stack(qS, 0, 2)
    proj_stack(kS, C, 4)
    # v: [key_tile, head, hd] with ones column scaled by 10 (folds the 0.1)
    nc.vector.memset(vv[:, :, :, HD], 10.0)
    for kt in range(16):
        pv = spp.tile([P, NQ], FP32, tag="sp", name="pv")[:, :512]
        for cc in range(2):
            nc.tensor.matmul(pv[:, :C], xT[cc][:, ts(kt, P)],
                             wT[cc][:, ds(2 * C, C)],
                             start=(cc == 0), stop=(cc == 1))
        nc.vector.tensor_copy(
            vv[:, kt, :, 0:HD],
            pv[:, :C].rearrange("p (h d) -> p h d", h=H))

    # ---------------- attention: head pairs ----------------
    for hA, hB in PAIRS:
        sA, rA = hA // 3, 32 * (hA % 3)
        sB, rB = hB // 3, 32 * (hB % 3)
        po = pop.tile([P, NQ], FP32, tag="po", name="po")
        natt = [0]

        def attnv(kt, E2):
            natt[0] += 1
            for c in range(2):
                nc.tensor.matmul(po[ds(0, HD + 1), ds(512 * c, 512)],
                                 vv[:, kt, hA, :],
                                 E2[:, 0, ds(512 * c, 512)],
                                 start=(natt[0] == 1), stop=(natt[0] == 16),
                                 skip_group_check=True)
                nc.tensor.matmul(po[ds(64, HD + 1), ds(512 * c, 512)],
                                 vv[:, kt, hB, :],
                                 E2[:, 1, ds(512 * c, 512)],
                                 start=(natt[0] == 1), stop=(natt[0] == 16),
                                 skip_group_check=True)

        for kt in range(16):
            spA = spp.tile([P, NQ], FP32, tag="sp", name="spA")
            spB = spp.tile([P, NQ], FP32, tag="sp", name="spB")
            for c in range(2):
                nc.tensor.matmul(spA[:, ds(512 * c, 512)],
                                 kS[sA][ds(rA, HD), ts(kt, P)],
                                 qS[sA][ds(rA, HD), ds(512 * c, 512)],
                                 start=True, stop=True)
                nc.tensor.matmul(spB[:, ds(512 * c, 512)],
                                 kS[sB][ds(rB, HD), ts(kt, P)],
                                 qS[sB][ds(rB, HD), ds(512 * c, 512)],
                                 start=True, stop=True)
            if kt in POOL_KTS:
                ea = eadjT[:, POOL_KTS.index(kt), :]
                E2 = work.tile([P, 2, NQ], BF16, tag="E2", name="E2", bufs=6)
                PtA = work.tile([P, NQ], BF16, tag="Pt", name="PtA", bufs=4)
                PtB = work.tile([P, NQ], BF16, tag="Pt", name="PtB", bufs=4)
                nc.scalar.activation(PtA[:], spA[:], AF.Exp)
                nc.gpsimd.tensor_tensor(E2[:, 0, :], PtA[:], ea, ALU.mult)
                nc.scalar.activation(PtB[:], spB[:], AF.Exp)
                nc.gpsimd.tensor_tensor(E2[:, 1, :], PtB[:], ea, ALU.mult)
            else:
                aj = adjF[:, kt, :]
                sm2 = work.tile([P, 2, NQ], FP32, tag="sm2", name="sm2", bufs=3)
                nc.vector.tensor_tensor(sm2[:, 0, :], spA[:], aj, ALU.add)
                nc.vector.tensor_tensor(sm2[:, 1, :], spB[:], aj, ALU.add)
                E2 = work.tile([P, 2, NQ], BF16, tag="E2", name="E2", bufs=6)
                nc.scalar.activation(E2[:], sm2[:], AF.Exp)
            attnv(kt, E2)
        # ---- normalize: rows base..base+31 by 1/(10*sum) ----
        # reciprocal on a DMA-spread [64,32] layout (DVE recip cost scales
        # with free size; [1,1024] is ~7.9us, this form ~0.5us total)
        cpA = work.tile([1, NQ], FP32, tag="cpA", name="cpA")
        cpB = work.tile([1, NQ], FP32, tag="cpB", name="cpB")
        nc.scalar.copy(cpA[:], po[ds(HD, 1), :])
        nc.vector.tensor_copy(cpB[:], po[ds(64 + HD, 1), :])
        dsp = work.tile([64, 32], FP32, tag="dsp", name="dsp")
        for i, cp in enumerate((cpA, cpB)):
            nc.sync.dma_start(dsp[ds(32 * i, 32), :],
                              cp[:, :].rearrange("o (p j) -> o p j", p=32))
        rc = work.tile([64, 32], FP32, tag="rc", name="rc")
        nc.vector.reciprocal(rc[:], dsp[:])
        rr = work.tile([2, NQ], FP32, tag="rr", name="rr")
        for i in range(2):
            nc.sync.dma_start(rr[ds(i, 1), :].rearrange("o (p j) -> o p j", p=32),
                              rc[ds(32 * i, 32), :])
        for i, (h, base) in enumerate(((hA, 0), (hB, 64))):
            bc = work.tile([HD, NQ], FP32, tag="bc", name="bc")
            nc.sync.dma_start(bc[:], rr[ds(i, 1), None, :].to_broadcast((1, HD, NQ)))
            home, chunk = 32 * (h % 4), h // 4
            nc.vector.tensor_tensor(attT[chunk][ds(home, HD), :],
                                    po[ds(base, HD), :], bc[:], ALU.mult)

    # ---------------- residual + out_proj ----------------
    for cc in range(2):
        nc.vector.tensor_tensor(attT[cc][:], attT[cc][:], xT[cc][:, 0:NQ],
                                ALU.add)
    for rt in range(8):
        pf = spp.tile([P, NQ], FP32, tag="sp", name="pf")[:, :C]
        for cc in range(2):
            nc.tensor.matmul(pf[:, :C], attT[cc][:, ts(rt, P)], owT[cc][:],
                             start=(cc == 0), stop=(cc == 1))
        osb = work.tile([P, C], FP32, tag="osb", name="osb")
        nc.vector.tensor_tensor(osb[:], pf[:, :C], outb_bc[:], ALU.add)
        nc.sync.dma_start(out_ext[ds(rt * P, P), :], osb[:])


def _run(nc, in_maps):
    from concourse.bass_utils import run_bass_kernel_spmd
    res = run_bass_kernel_spmd(nc, in_maps, core_ids=list(range(8)))
    return res.results


def make_in_maps(x, adj, qkv_w, out_w, out_b):
    x = np.asarray(x, np.float32)
    adj = np.asarray(adj, np.float32)
    w = np.asarray(qkv_w, np.float32).copy()
    w[:C] *= SCALE                       # fold 1/sqrt(HD) into q weights
    wt = np.ascontiguousarray(w.T).astype(BF16NP)
    owt = np.ascontiguousarray(np.asarray(out_w, np.float32).T).astype(BF16NP)
    outb = np.ascontiguousarray(
        np.broadcast_to(np.asarray(out_b, np.float32), (P, C)))
    in_maps = []
    for c in range(8):
        b, half = divmod(c, 2)
        xb = np.roll(x[b], -half * NQ, axis=0)
        xt = np.ascontiguousarray(xb.T).astype(BF16NP)          # [C, N]
        aj = np.roll(adj[half * NQ:(half + 1) * NQ, :], -half * NQ, axis=1)
        ajf = np.ascontiguousarray(
            aj.T.reshape(16, P, NQ).transpose(1, 0, 2))          # [P, 16, NQ]
        in_maps.append({
            "xt": xt, "adjf": ajf,
            "qkv_wt": wt, "out_wt": owt, "out_b": outb,
        })
    return in_maps


def kernel(x, x0, adj, qkv_w, out_w, out_b, pos_w, pos_b):
    """Full-input, full-output entry point.  x0/pos_w/pos_b are dead in the
    reference computation and are ignored."""
    if "nc" not in _CACHED:
        _CACHED["nc"] = build_kernel(repeat=1)
    nc = _CACHED["nc"]
    in_maps = make_in_maps(x, adj, qkv_w, out_w, out_b)
    results = _run(nc, in_maps)
    out = np.empty((B, N, C), np.float32)
    for c in range(8):
        b, half = divmod(c, 2)
        out[b, half * NQ:(half + 1) * NQ, :] = results[c]["out"]
    return out
